# revision 2
# baseline (speedup 1.0000x reference)
"""EuclideanVisitEncoder Trainium2 kernel (CCE-accumulate design).

Masked-mean embedding lookup:
    out[n, :] = mean over c of emb[code_ids[n, c]] where code_ids[n, c] >= 0

Strategy (8 NeuronCores, SPMD data-parallel over visits):
  - Shard the 16384 visits into 8 x 2048; replicate the table. Host pads the
    table with one zero row; negative ids are remapped on-device to that row
    via one unsigned-min op, so the gather needs no masking.
  - Per 128-visit block, the 64 code columns are gathered with 64 indirect
    DMAs (one offset per partition — the only reliable form of this HW's
    SWDGE indirect ucode) that ACCUMULATE via the SDMA CCE-add datapath into
    a single [128, 64] f32 accumulator. All qPoolDynamic descriptors drain
    in ring-FIFO order per SDMA engine and each partition is served by
    exactly one engine, so same-destination adds cannot race and the
    inter-gather semaphore waits Tile inserts are redundant — they are
    stripped post-hoc, leaving the Pool engine free to issue gathers
    back-to-back at the SWDGE generation rate (the kernel's bottleneck).
  - Block b's last gather is queue instruction 64b+63, which lands on DMA
    lane (64b+63)%8 == 7 with cumulative value 128(b+1); ring order makes a
    single wait on that value a complete barrier for the block, so the
    per-block recip-scale needs exactly one sync wait (the walrus build
    rejects more than one).
  - No on-chip reduction remains: DVE only preps ids/counts and scales.
"""

import sys

for _p in ("/opt/trn_rl_repo",):
    if _p not in sys.path:
        sys.path.append(_p)

import numpy as np

import concourse.bass as bass
import concourse.mybir as mybir
import concourse.tile as tile
from concourse.bass_utils import run_bass_kernel_spmd
from concourse.tile_rust import add_dep_helper

NUM_CODES = 100000
DIM = 64
N_VISITS = 16384
MAX_CODES = 64
N_CORES = 8
VPC = N_VISITS // N_CORES  # visits per core
P = 128
N_BLOCKS = VPC // P
ZERO_ROW = NUM_CODES  # host-padded all-zero table row
TAB_ROWS = NUM_CODES + 1
N_LANES = 8


def build_bass():
    nc = bass.Bass()
    ids = nc.declare_dram_parameter("ids", [VPC, MAX_CODES], mybir.dt.int32, isOutput=False)
    table = nc.declare_dram_parameter("table", [TAB_ROWS, DIM], mybir.dt.float32, isOutput=False)
    out = nc.declare_dram_parameter("out", [VPC, DIM], mybir.dt.float32, isOutput=True)

    gather_names = []
    mul_names = {}

    with tile.TileContext(nc) as tc:
        with tc.tile_pool(name="io_pool", bufs=1) as io_pool:
            # Bulk load of all ids: SBUF layout [p, block, code], visit = 128b + p.
            ids_all = io_pool.tile([P, N_BLOCKS * MAX_CODES], mybir.dt.int32)
            nc.sync.dma_start(
                out=ids_all[:].rearrange("p (b c) -> p b c", c=MAX_CODES),
                in_=ids.rearrange("(b p) c -> p b c", p=P),
            )
            # Negative ids -> ZERO_ROW: as uint32 a negative id is huge, so
            # unsigned min(id, ZERO_ROW) clamps exactly those. This DVE op
            # also folds the ids-load DMA completion into the DVE stream.
            safe_all = io_pool.tile([P, N_BLOCKS * MAX_CODES], mybir.dt.uint32)
            nc.vector.tensor_scalar_min(
                safe_all[:], ids_all[:].bitcast(mybir.dt.uint32), ZERO_ROW
            )
            validf_all = io_pool.tile([P, N_BLOCKS * MAX_CODES], mybir.dt.float32)
            nc.vector.tensor_scalar(
                out=validf_all[:],
                in0=ids_all[:],
                scalar1=0,
                scalar2=None,
                op0=mybir.AluOpType.is_ge,
            )
            counts_all = io_pool.tile([P, N_BLOCKS], mybir.dt.float32)
            nc.vector.reduce_sum(
                out=counts_all[:],
                in_=validf_all[:].rearrange("p (b c) -> p b c", c=MAX_CODES),
                axis=mybir.AxisListType.X,
            )
            nc.vector.tensor_scalar_max(counts_all[:], counts_all[:], 1.0)
            recip_all = io_pool.tile([P, N_BLOCKS], mybir.dt.float32)
            nc.vector.reciprocal(recip_all[:], counts_all[:])

            # Accumulators, zeroed once (CCE-add needs zeroed destinations).
            sums_all = io_pool.tile([P, N_BLOCKS * DIM], mybir.dt.float32)
            zs = nc.vector.memset(sums_all[:], 0.0)

            out_all = io_pool.tile([P, N_BLOCKS * DIM], mybir.dt.float32)

            for b in range(N_BLOCKS):
                for c in range(MAX_CODES):
                    gi = nc.gpsimd.indirect_dma_start(
                        out=sums_all[:, b * DIM : (b + 1) * DIM],
                        out_offset=None,
                        in_=table[:],
                        in_offset=bass.IndirectOffsetOnAxis(
                            ap=safe_all[:, b * MAX_CODES + c : b * MAX_CODES + c + 1],
                            axis=0,
                        ),
                        compute_op=mybir.AluOpType.add,
                    )
                    gather_names.append(gi.ins.name)
                mi = nc.vector.tensor_scalar_mul(
                    out_all[:, b * DIM : (b + 1) * DIM],
                    sums_all[:, b * DIM : (b + 1) * DIM],
                    recip_all[:, b : b + 1],
                )
                add_dep_helper(mi.ins, zs.ins, False, "order after zero")
                mul_names[mi.ins.name] = b

            nc.sync.dma_start(
                out=out.rearrange("(b p) d -> p b d", p=P),
                in_=out_all[:].rearrange("p (b d) -> p b d", d=DIM),
            )
            # Dummy DVE write WAR-ordered after the final store: folds the
            # store's completion into the DVE stream for the tail drain.
            nc.vector.memset(out_all[:1, :1], 0.0)

    _apply_fifo_sync(nc, gather_names, mul_names)
    _strip_redundant_dma_waits(nc)
    return nc


def _apply_fifo_sync(nc, gather_names, mul_names):
    """Rewrite DMASW waits using qPoolDynamic ring-FIFO ordering.

    All indirect gathers share one SWDGE queue; each SDMA engine drains that
    queue's descriptors in order and each SBUF partition is served by exactly
    one engine. Hence (a) same-destination CCE-adds cannot race and the
    inter-gather waits are redundant, and (b) a wait on the lane semaphore of
    queue instruction t at its cumulative value covers every queue
    instruction <= t. Gather t (0-based) lands on lane t%8 with cumulative
    value 16*(t//8+1); block b's last gather is t=64b+63 -> lane 7, value
    128(b+1).
    """
    gset = set(gather_names)
    order = {n: t for t, n in enumerate(gather_names)}
    last_t = len(gather_names) - 1
    last_lane = f"DMASW{last_t % N_LANES}"
    for f in nc.m.functions:
        for blk in f.blocks:
            for i in blk.instructions:
                si = i.sync_info
                if not si:
                    continue
                if i.name in gset:
                    # keep only non-DMASW waits (first gather keeps its DVE
                    # wait on the prep/zero; later ones drop to zero waits)
                    si.on_wait = [
                        w
                        for w in si.on_wait
                        if not (w.ant_name or "").startswith("DMASW")
                    ]
                    i.sync_info = si
                elif type(i).__name__ == "InstDrain":
                    si.on_wait = [
                        w
                        for w in si.on_wait
                        if not (w.ant_name or "").startswith("DMASW")
                        or (w.ant_name or "").startswith(last_lane + "_")
                    ]
                    i.sync_info = si


def _strip_redundant_dma_waits(nc):
    """Reduce every instruction to at most one sync wait.

    This toolchain's walrus build rejects instructions with more than one
    sync wait ("Too many sync wait commands"), but Tile's semaphore pass is
    not transitive across processors and happily emits several. Two sound
    reductions (semaphores are monotonic counters, engines issue in order):

    1. Same-stream: if an earlier instruction on the same engine stream
       already waited (s >= v'), v' >= v, the condition still holds when a
       later instruction on that stream issues - drop (s >= v).
    2. Cross-engine: an engine sem E >= t means the first t instructions of
       E's stream completed, hence started, hence every wait they carry held.
       If those waits cover (s >= v), a kept wait (E >= t) implies it.
    """
    insts = [i for f in nc.m.functions for blk in f.blocks for i in blk.instructions]
    eng_sem_insts: dict[str, list] = {}
    for i in insts:
        si = i.sync_info
        if not si:
            continue
        for u in si.on_update:
            name = u.ant_name or ""
            if name.startswith(("DVE", "Act", "PE", "Pool", "SP")) and "DMA" not in name:
                eng_sem_insts.setdefault(name, []).append((u.update_value, i))

    def implied(kept_name: str, kept_val: int, s_name: str, s_val: int) -> bool:
        stream = eng_sem_insts.get(kept_name)
        if not stream:
            return False
        tot = 0
        for upd, i in stream:
            if tot >= kept_val:
                break
            tot += upd
            si = i.sync_info
            for w in si.on_wait if si else []:
                if w.ant_name == s_name and w.wait_value >= s_val:
                    return True
        return False

    observed_seq: dict = {}
    observed_eng: dict = {}
    own_updates: dict = {}
    for i in insts:
        si = i.sync_info
        if not si:
            continue
        eng = i.engine
        is_dma = type(i).__name__ in ("InstDMACopy", "InstNoOp")
        seen_seq = observed_seq.setdefault(eng, {})
        seen_eng = observed_eng.setdefault(eng, {})
        seen = seen_seq if is_dma else seen_eng
        waits = list(si.on_wait)
        if len(waits) > 1:
            remaining = [w for w in waits if seen.get(w.ant_name, -1) < w.wait_value]
            if len(remaining) > 1:
                for kept in remaining:
                    others = [w for w in remaining if w is not kept]
                    if all(
                        implied(kept.ant_name, kept.wait_value, w.ant_name, w.wait_value)
                        for w in others
                    ):
                        remaining = [kept]
                        break
                else:
                    raise RuntimeError(
                        f"{type(i).__name__} {i.name} ({eng}) has waits "
                        f"{[(w.ant_name, w.wait_value) for w in remaining]}, "
                        f"cannot reduce to one"
                    )
            si.on_wait = remaining
            i.sync_info = si
            waits = remaining
        for w in waits:
            if seen_eng.get(w.ant_name, -1) < w.wait_value:
                seen_eng[w.ant_name] = w.wait_value
            if is_dma and seen_seq.get(w.ant_name, -1) < w.wait_value:
                seen_seq[w.ant_name] = w.wait_value
        if not is_dma:
            cum = own_updates.setdefault(eng, {})
            for u in si.on_update:
                name = u.ant_name or ""
                cum[name] = cum.get(name, 0) + u.update_value
                if seen_eng.get(name, -1) < cum[name]:
                    seen_eng[name] = cum[name]


_NC_CACHE = None


def _get_nc():
    global _NC_CACHE
    if _NC_CACHE is None:
        _NC_CACHE = build_bass()
    return _NC_CACHE


def kernel(code_ids: np.ndarray, emb_weight: np.ndarray) -> np.ndarray:
    assert code_ids.shape == (N_VISITS, MAX_CODES)
    assert emb_weight.shape == (NUM_CODES, DIM)
    ids32 = np.ascontiguousarray(code_ids.astype(np.int32))
    table = np.concatenate(
        [emb_weight.astype(np.float32), np.zeros((1, DIM), np.float32)], axis=0
    )
    in_maps = [
        {"ids": ids32[k * VPC : (k + 1) * VPC], "table": table} for k in range(N_CORES)
    ]
    nc = _get_nc()
    res = run_bass_kernel_spmd(nc, in_maps, list(range(N_CORES)))
    return np.concatenate([res.results[k]["out"] for k in range(N_CORES)], axis=0)


# revision 3
# speedup vs baseline: 1.5572x; 1.5572x over previous
"""EuclideanVisitEncoder Trainium2 kernel.

Masked-mean embedding lookup:
    out[n, :] = mean over c of emb[code_ids[n, c]] where code_ids[n, c] >= 0
    (visits with zero valid codes produce a zero vector)

Strategy (8 NeuronCores, SPMD data-parallel over visits):
  - Shard the 16384 visits into 8 x 2048; replicate the 25.6MB table.
  - Host pads the table with one zero row at index NUM_CODES; negative ids
    are remapped on-device to that row via a single unsigned-min op, so the
    gather needs no masking, no memset and no bounds check.
  - Per 128-visit block, ONE indirect DMA (SWDGE) gathers all 128*64
    embedding rows (256B each) into a [128, 4096] SBUF tile laid out
    [visit partition, code, dim].
  - Vector engine reduces over the code axis (strided AP), computes
    1/max(count,1) from the ids, scales, and the result is stored.
No cross-core communication is needed.
"""

import sys

for _p in ("/opt/trn_rl_repo",):
    if _p not in sys.path:
        sys.path.append(_p)

import numpy as np

import concourse.bass as bass
import concourse.mybir as mybir
import concourse.tile as tile
from concourse.bass_utils import run_bass_kernel_spmd
from concourse.tile_rust import add_dep_helper

NUM_CODES = 100000
DIM = 64
N_VISITS = 16384
MAX_CODES = 64
N_CORES = 8
VPC = N_VISITS // N_CORES  # visits per core
P = 128
N_BLOCKS = VPC // P
ZERO_ROW = NUM_CODES  # host-padded all-zero table row
TAB_ROWS = NUM_CODES + 1


def build_bass():
    nc = bass.Bass()
    ids = nc.declare_dram_parameter("ids", [VPC, MAX_CODES], mybir.dt.int32, isOutput=False)
    table = nc.declare_dram_parameter("table", [TAB_ROWS, DIM], mybir.dt.float32, isOutput=False)
    out = nc.declare_dram_parameter("out", [VPC, DIM], mybir.dt.float32, isOutput=True)

    with tile.TileContext(nc) as tc:
        with (
            tc.tile_pool(name="io_pool", bufs=1) as io_pool,
            tc.tile_pool(name="gath_pool", bufs=6) as gath_pool,
            tc.tile_pool(name="small_pool", bufs=4) as small_pool,
        ):
            # One bulk load of all ids: SBUF layout [p, block, code] where
            # visit = block*128 + p. One bulk store of all outputs at the end.
            ids_all = io_pool.tile([P, N_BLOCKS * MAX_CODES], mybir.dt.int32)
            nc.sync.dma_start(
                out=ids_all[:].rearrange("p (b c) -> p b c", c=MAX_CODES),
                in_=ids.rearrange("(b p) c -> p b c", p=P),
            )
            out_all = io_pool.tile([P, N_BLOCKS * DIM], mybir.dt.float32)

            # All per-id prep in whole-tensor DVE ops (keeps every DMA at
            # <=1 sync wait: the HW DMA instruction has a single wait slot).
            # Negative ids -> ZERO_ROW: as uint32 a negative id is huge,
            # so unsigned min(id, ZERO_ROW) clamps exactly those.
            safe_all = io_pool.tile([P, N_BLOCKS * MAX_CODES], mybir.dt.uint32)
            nc.vector.tensor_scalar_min(
                safe_all[:], ids_all[:].bitcast(mybir.dt.uint32), ZERO_ROW
            )
            validf_all = io_pool.tile([P, N_BLOCKS * MAX_CODES], mybir.dt.float32)
            nc.vector.tensor_scalar(
                out=validf_all[:],
                in0=ids_all[:],
                scalar1=0,
                scalar2=None,
                op0=mybir.AluOpType.is_ge,
            )
            counts_all = io_pool.tile([P, N_BLOCKS], mybir.dt.float32)
            nc.vector.reduce_sum(
                out=counts_all[:],
                in_=validf_all[:].rearrange("p (b c) -> p b c", c=MAX_CODES),
                axis=mybir.AxisListType.X,
            )
            nc.vector.tensor_scalar_max(counts_all[:], counts_all[:], 1.0)
            recip_all = io_pool.tile([P, N_BLOCKS], mybir.dt.float32)
            nc.vector.reciprocal(recip_all[:], counts_all[:])

            GBUFS = 6
            lane_sc = io_pool.tile([P, 8], mybir.dt.float32)
            red_insts = []
            for b in range(N_BLOCKS):
                gath = gath_pool.tile([P, MAX_CODES * DIM], mybir.dt.float32)
                # Pool-sequencer absorber: carries the WAR wait (the b-GBUFS
                # reduce freed this slot) so every gather below keeps only
                # its single DMA-lane wait (HW allows one wait per DMA).
                nop = None
                if b >= GBUFS:
                    nop = nc.engines[mybir.EngineType.Pool].nop(
                        nofuse=True, hint=f"ab{b}"
                    )
                    add_dep_helper(
                        nop.ins, red_insts[b - GBUFS].ins, True, "absorb WAR"
                    )
                # The multi-offset form of indirect_dma_start is broken in
                # this HW ucode (only one offset per partition lands
                # correctly), so gather one code column per instruction.
                for c in range(MAX_CODES):
                    gi = nc.gpsimd.indirect_dma_start(
                        out=gath[:, c * DIM : (c + 1) * DIM],
                        out_offset=None,
                        in_=table[:],
                        in_offset=bass.IndirectOffsetOnAxis(
                            ap=safe_all[:, b * MAX_CODES + c : b * MAX_CODES + c + 1],
                            axis=0,
                        ),
                    )
                    if nop is not None and c == 0:
                        add_dep_helper(gi.ins, nop.ins, False, "order after absorber")

                # Fold the last 8 gathers' completion (all 8 DMASW lanes at
                # their newest values) into the DVE stream one wait at a
                # time, so the reduce below needs no waits of its own.
                for j in range(8):
                    cc = (MAX_CODES - 8 + j) * DIM
                    nc.vector.tensor_copy(lane_sc[:1, j : j + 1], gath[:1, cc : cc + 1])
                sums = small_pool.tile([P, DIM], mybir.dt.float32)
                ri = nc.vector.reduce_sum(
                    out=sums[:],
                    in_=gath[:].rearrange("p (c d) -> p d c", d=DIM),
                    axis=mybir.AxisListType.X,
                )
                red_insts.append(ri)
                nc.vector.tensor_scalar_mul(
                    out_all[:, b * DIM : (b + 1) * DIM], sums[:], recip_all[:, b : b + 1]
                )

            nc.sync.dma_start(
                out=out.rearrange("(b p) d -> p b d", p=P),
                in_=out_all[:].rearrange("p (b d) -> p b d", d=DIM),
            )
            # Dummy DVE write ordered after the final store (WAR): folds the
            # store's completion into the DVE stream so the kernel-tail drain
            # can be reduced to a single DVE wait below.
            nc.vector.memset(out_all[:1, :1], 0.0)
    _strip_redundant_dma_waits(nc)
    return nc


def _strip_redundant_dma_waits(nc):
    """Reduce every instruction to at most one sync wait.

    This toolchain's walrus build rejects instructions with more than one
    sync wait ("Too many sync wait commands"), but Tile's semaphore pass is
    not transitive across processors and happily emits several. Two sound
    reductions (semaphores are monotonic counters, engines issue in order):

    1. Same-stream: if an earlier instruction on the same engine stream
       already waited (s >= v'), v' >= v, the condition still holds when a
       later instruction on that stream issues - drop (s >= v).
    2. Cross-engine: an engine sem E >= t means the first t instructions of
       E's stream completed, hence started, hence every wait they carry held.
       If those waits cover (s >= v), a kept wait (E >= t) implies it.
    """
    insts = [i for f in nc.m.functions for blk in f.blocks for i in blk.instructions]
    # Per-sem ordered update streams for single-proc engine sems.
    eng_sem_insts: dict[str, list] = {}
    for i in insts:
        si = i.sync_info
        if not si:
            continue
        for u in si.on_update:
            name = u.ant_name or ""
            if name.startswith(("DVE", "Act", "PE", "Pool", "SP")) and "DMA" not in name:
                eng_sem_insts.setdefault(name, []).append((u.update_value, i))

    def implied(kept_name: str, kept_val: int, s_name: str, s_val: int) -> bool:
        stream = eng_sem_insts.get(kept_name)
        if not stream:
            return False
        tot = 0
        for upd, i in stream:
            if tot >= kept_val:
                break
            tot += upd
            si = i.sync_info
            for w in si.on_wait if si else []:
                if w.ant_name == s_name and w.wait_value >= s_val:
                    return True
        return False

    # Observed (sem -> value) per engine stream so far, in block order.
    # Two maps per engine: waits performed at the sequencer (DMA issue) are
    # ordered before everything later on the stream; waits performed at the
    # engine (compute) only order against later engine-executed work - a
    # later DMA's sequencer-level wait may run ahead of queued compute.
    observed_seq: dict = {}
    observed_eng: dict = {}
    # Cumulative sem updates issued by each engine's own in-order (non-DMA)
    # instructions: a later instruction on that engine sees at least that
    # value (updates fire at completion, completion precedes later issue).
    own_updates: dict = {}
    for i in insts:
        si = i.sync_info
        if not si:
            continue
        eng = i.engine
        # DMAs and nops take their waits at the sequencer; compute
        # instructions wait in the engine queue.
        is_dma = type(i).__name__ in ("InstDMACopy", "InstNoOp")
        seen_seq = observed_seq.setdefault(eng, {})
        seen_eng = observed_eng.setdefault(eng, {})
        seen = seen_seq if is_dma else seen_eng
        waits = list(si.on_wait)
        if len(waits) > 1:
            # Rule 1: drop waits already observed by this engine stream.
            remaining = [
                w for w in waits if seen.get(w.ant_name, -1) < w.wait_value
            ]
            if len(remaining) > 1:
                # Rule 2: find one wait implying all the others.
                for kept in remaining:
                    others = [w for w in remaining if w is not kept]
                    if all(
                        implied(kept.ant_name, kept.wait_value, w.ant_name, w.wait_value)
                        for w in others
                    ):
                        remaining = [kept]
                        break
                else:
                    raise RuntimeError(
                        f"{type(i).__name__} {i.name} ({eng}) has waits "
                        f"{[(w.ant_name, w.wait_value) for w in remaining]}, "
                        f"cannot reduce to one"
                    )
            si.on_wait = remaining
            i.sync_info = si
            waits = remaining
        for w in waits:
            if seen_eng.get(w.ant_name, -1) < w.wait_value:
                seen_eng[w.ant_name] = w.wait_value
            if is_dma and seen_seq.get(w.ant_name, -1) < w.wait_value:
                seen_seq[w.ant_name] = w.wait_value
        if not is_dma:
            cum = own_updates.setdefault(eng, {})
            for u in si.on_update:
                name = u.ant_name or ""
                cum[name] = cum.get(name, 0) + u.update_value
                if seen_eng.get(name, -1) < cum[name]:
                    seen_eng[name] = cum[name]


_NC_CACHE = None


def _get_nc():
    global _NC_CACHE
    if _NC_CACHE is None:
        _NC_CACHE = build_bass()
    return _NC_CACHE


def kernel(code_ids: np.ndarray, emb_weight: np.ndarray) -> np.ndarray:
    assert code_ids.shape == (N_VISITS, MAX_CODES)
    assert emb_weight.shape == (NUM_CODES, DIM)
    ids32 = np.ascontiguousarray(code_ids.astype(np.int32))
    table = np.concatenate(
        [emb_weight.astype(np.float32), np.zeros((1, DIM), np.float32)], axis=0
    )
    in_maps = [
        {"ids": ids32[k * VPC : (k + 1) * VPC], "table": table} for k in range(N_CORES)
    ]
    nc = _get_nc()
    res = run_bass_kernel_spmd(nc, in_maps, list(range(N_CORES)))
    return np.concatenate([res.results[k]["out"] for k in range(N_CORES)], axis=0)

